# revision 29
# baseline (speedup 1.0000x reference)
"""Born-Mayer-Huggins pairwise energy + forces on 8 Trainium2 NeuronCores.

Row-shards the [N,N] pair matrices across 8 cores (N/8 rows each); each core
computes its force rows and per-row energy sums on device; host concatenates
forces and sums the energy.

Math (per pair i,j, minimum image under cubic PBC, box L):
    d_c   = (xj - xi)_c - L*round((xj - xi)_c / L)       (signed, = -dx_mi)
    r2    = sum_c d_c^2 ;  cl = max(r2, md^2) ;  r = sqrt(cl)
    t     = exp((sigma - r)/rho)
    e     = A*t - C*cl^-3 + D*cl^-4                      (cutoff mask dropped:
                                                          tail error ~3e-7 rel)
    g     = (A*t/rho)/r - 6*C*cl^-4 + 8*D*cl^-5
    F_i,c = -sum_j g*(r2>md^2)*d_c ;  E = (sum e - sum_diag e)/2
Diagonal energy terms (r clamped to md) are subtracted exactly per row using
the diagonal elements of A,C,D,rho,sigma.

Implementation: per 128-row tile, the 4096 pair columns are processed in
chunks as a two-stage software pipeline — "front" (minimum image, r^2,
ln/exp power chain) of chunk ch is emitted interleaved with "back"
(energy/force combine + row reductions) of chunk ch-1, so each in-order
engine stream has independent work to fill cross-engine waits.  The
minimum-image shift tiles are produced on GPSIMD, transcendentals on the
scalar engine (one activation-table set: ln/exp/square/relu), everything
else on the vector engine with fused scalar_tensor_tensor ops + free
row-sum accumulators.
"""

import numpy as np

N = 4096
NCORES = 8
RPC = N // NCORES       # rows per core
P = 128                 # SBUF partitions
RT = RPC // P           # row tiles per core
FC = 512                # free-dim chunk
NCH = N // FC           # chunks per row

_cache = {}


def _build(L, md):
    import concourse.bass as bass
    import concourse.tile as tile
    from concourse import bacc, mybir
    from contextlib import ExitStack

    AT = mybir.ActivationFunctionType
    OP = mybir.AluOpType
    F32 = mybir.dt.float32
    BF16 = mybir.dt.bfloat16
    I32 = mybir.dt.int32
    md2 = float(md) * float(md)
    relu_clamp = md2 == 1.0
    half = float(L) / 2.0

    nc = bacc.Bacc("TRN2", target_bir_lowering=False, debug=False)
    Ad = nc.dram_tensor("Ash", [RPC, N], F32, kind="ExternalInput").ap()
    Cd = nc.dram_tensor("Csh", [RPC, N], F32, kind="ExternalInput").ap()
    Dd = nc.dram_tensor("Dsh", [RPC, N], F32, kind="ExternalInput").ap()
    Rd = nc.dram_tensor("Rsh", [RPC, N], F32, kind="ExternalInput").ap()
    Sd = nc.dram_tensor("Ssh", [RPC, N], F32, kind="ExternalInput").ap()
    xTd = nc.dram_tensor("xallT", [3, N], F32, kind="ExternalInput").ap()
    xod = nc.dram_tensor("xown", [RPC, 3], F32, kind="ExternalInput").ap()
    dgd = nc.dram_tensor("diagd", [RPC, 5], F32, kind="ExternalInput").ap()
    outd = nc.dram_tensor("out", [RPC, 4], F32, kind="ExternalOutput").ap()

    with tile.TileContext(nc) as tc, ExitStack() as ctx:
        singles = ctx.enter_context(tc.tile_pool(name="singles", bufs=1))
        inpool = ctx.enter_context(tc.tile_pool(name="inp", bufs=3))
        vp = ctx.enter_context(tc.tile_pool(name="vp", bufs=2))
        gp = ctx.enter_context(tc.tile_pool(name="gp", bufs=2))
        ap2 = ctx.enter_context(tc.tile_pool(name="ap2", bufs=2))
        psum = ctx.enter_context(tc.tile_pool(name="ps", bufs=2, space="PSUM"))
        colp = ctx.enter_context(tc.tile_pool(name="col", bufs=2))

        # coordinates of all atoms, broadcast along partitions: [128, 3, N].
        # Loaded lazily per column chunk during the first row tile so early
        # compute is not gated on the full 6 MB broadcast.
        XJ = singles.tile([P, 3, N], F32)

        def xj_load(ch):
            cs = ch * FC
            for c in range(3):
                nc.sync.dma_start(
                    XJ[:, c, cs : cs + FC],
                    xTd[c : c + 1, cs : cs + FC].to_broadcast([P, FC]),
                )

        def w(tag, pool=None, dt=F32, bufs=None):
            pl = pool or vp
            return pl.tile([P, FC], dt, tag=tag, name=tag, bufs=bufs)

        # per-partition coordinate columns and thresholds, all row tiles
        cols = []
        for rt in range(RT):
            rs = rt * P
            xi = colp.tile([P, 3], F32, tag="xi", name="xi", bufs=RT)
            nc.gpsimd.dma_start(xi[:, :], xod[rs : rs + P, :])
            xp = colp.tile([P, 3], F32, tag="xp", name="xp", bufs=RT)
            nc.vector.tensor_scalar(xp[:, :], xi[:, :], half, None, OP.add)
            xm = colp.tile([P, 3], F32, tag="xm", name="xm", bufs=RT)
            nc.vector.tensor_scalar(xm[:, :], xi[:, :], -half, None, OP.add)
            xn = colp.tile([P, 3], F32, tag="xn", name="xn", bufs=RT)
            nc.vector.tensor_scalar(xn[:, :], xi[:, :], -1.0, None, OP.mult)
            xoff = colp.tile([P, 3], F32, tag="xoff", name="xoff", bufs=RT)
            nc.vector.tensor_scalar(xoff[:, :], xi[:, :], -1.0, L, OP.mult, OP.add)
            cols.append((xp, xm, xn, xoff))

        for rt in range(RT):
            rs = rt * P
            xp, xm, xn, xoff = cols[rt]

            ecol = colp.tile([P, NCH], F32, tag="ecol")
            fcol = colp.tile([P, 3 * NCH], F32, tag="fcol")

            def pool_coords(ch):
                # minimum-image shift tiles (POOL only), via the f32->int32
                # RNE conversion: k = rint((xj - xi + L)/L) in {0,1,2},
                # s = L*(1-k) in {+L, 0, -L}
                cs = ch * FC
                out = []
                for c in range(3):
                    XJc = XJ[:, c, cs : cs + FC]
                    k = w("k", gp, dt=I32, bufs=1)
                    nc.gpsimd.tensor_scalar(
                        k[:], XJc, xoff[:, c : c + 1], 1.0 / L, OP.add, OP.mult
                    )
                    s = w(f"s{c}", gp)
                    nc.gpsimd.tensor_scalar(s[:], k[:], -L, L, OP.mult, OP.add)
                    out.append(s)
                return out

            def front(ch, ss):
                cs = ch * FC
                Ri = w("R", inpool)
                nc.sync.dma_start(Ri[:], Rd[rs : rs + P, cs : cs + FC])
                ds = []
                sqs = []
                for c in range(3):
                    XJc = XJ[:, c, cs : cs + FC]
                    d = w(f"d{c}")
                    nc.vector.scalar_tensor_tensor(
                        d[:], XJc, xn[:, c : c + 1], ss[c][:], OP.add, OP.add
                    )
                    sq = w("sq0", ap2) if c == 0 else w(f"sq{c}", psum)
                    nc.scalar.activation(sq[:], d[:], AT.Square)
                    ds.append(d)
                    sqs.append(sq)
                r2a = w("r2a")
                nc.vector.tensor_add(r2a[:], sqs[0][:], sqs[1][:])
                r2 = w("r2")
                nc.vector.tensor_add(r2[:], r2a[:], sqs[2][:])
                if relu_clamp:
                    # ln(max(r2,1)) == relu(ln(r2)); ln(0)=-inf -> relu -> 0
                    lr0 = w("lr0", ap2)
                    nc.scalar.activation(lr0[:], r2[:], AT.Ln)
                    lr = w("lrc", ap2)
                    nc.scalar.activation(lr[:], lr0[:], AT.Relu)
                else:
                    cl = w("lr0", ap2)
                    nc.vector.tensor_scalar(cl[:], r2[:], md2, None, OP.max)
                    lr = w("lrc", ap2)
                    nc.scalar.activation(lr[:], cl[:], AT.Ln)
                r_ = w("r_", ap2)
                nc.scalar.activation(r_[:], lr[:], AT.Exp, scale=0.5)
                rinv = w("rinv", ap2)
                nc.scalar.activation(rinv[:], lr[:], AT.Exp, scale=-0.5)
                r2i = w("r2i", ap2)
                nc.scalar.activation(r2i[:], lr[:], AT.Exp, scale=-1.0)
                r6i = w("r6i", ap2)
                nc.scalar.activation(r6i[:], lr[:], AT.Exp, scale=-3.0)
                r8i = w("r8i", ap2)
                nc.scalar.activation(r8i[:], lr[:], AT.Exp, scale=-4.0)
                lnr = w("lnr", ap2)
                nc.scalar.activation(lnr[:], Ri[:], AT.Ln)
                rhoi = w("rhoi", ap2)
                nc.scalar.activation(rhoi[:], lnr[:], AT.Exp, scale=-1.0)
                return dict(ds=ds, r2=r2, r_=r_, rinv=rinv, r2i=r2i,
                            r6i=r6i, r8i=r8i, rhoi=rhoi)

            def back(ch, f):
                cs = ch * FC
                Ai = w("A", inpool)
                nc.sync.dma_start(Ai[:], Ad[rs : rs + P, cs : cs + FC])
                Ci = w("C", inpool)
                nc.sync.dma_start(Ci[:], Cd[rs : rs + P, cs : cs + FC])
                Di = w("D", inpool)
                nc.sync.dma_start(Di[:], Dd[rs : rs + P, cs : cs + FC])
                Si = w("S", inpool)
                nc.sync.dma_start(Si[:], Sd[rs : rs + P, cs : cs + FC])

                diff = w("v1")
                nc.vector.scalar_tensor_tensor(
                    diff[:], f["r_"][:], -1.0, Si[:], OP.mult, OP.add
                )
                arg = w("arg", ap2)
                nc.vector.tensor_mul(arg[:], diff[:], f["rhoi"][:])
                t_ = w("t", psum)
                nc.scalar.activation(t_[:], arg[:], AT.Exp)

                At = w("v2")
                nc.vector.tensor_mul(At[:], Ai[:], t_[:])
                P1a = w("v1")
                nc.vector.tensor_mul(P1a[:], At[:], f["rhoi"][:])
                P1 = w("v3")
                nc.vector.tensor_mul(P1[:], P1a[:], f["rinv"][:])
                Du = w("Du", gp)
                nc.gpsimd.tensor_tensor(Du[:], Di[:], f["r2i"][:], OP.mult)
                CDe = w("v4")
                nc.vector.scalar_tensor_tensor(
                    CDe[:], Du[:], -1.0, Ci[:], OP.mult, OP.add
                )
                pe = w("pe", gp)
                nc.gpsimd.tensor_tensor(pe[:], f["r6i"][:], CDe[:], OP.mult)
                junk = w("junk", bufs=1)
                nc.vector.scalar_tensor_tensor(
                    junk[:], pe[:], -1.0, At[:], OP.mult, OP.add,
                    accum_out=ecol[:, ch : ch + 1],
                )
                CDg = w("v2")
                nc.vector.scalar_tensor_tensor(
                    CDg[:], Du[:], -4.0 / 3.0, Ci[:], OP.mult, OP.add
                )
                prod = w("v4")
                nc.vector.tensor_mul(prod[:], f["r8i"][:], CDg[:])
                g = w("v1")
                nc.vector.scalar_tensor_tensor(
                    g[:], prod[:], -6.0, P1[:], OP.mult, OP.add
                )
                gm = w("v3")
                nc.vector.scalar_tensor_tensor(
                    gm[:], f["r2"][:], md2, g[:], OP.is_gt, OP.mult
                )
                for c in range(3):
                    junk2 = w("junk", bufs=1)
                    nc.vector.scalar_tensor_tensor(
                        junk2[:], gm[:], 0.0, f["ds"][c][:], OP.add, OP.mult,
                        accum_out=fcol[:, c * NCH + ch : c * NCH + ch + 1],
                    )

            # two-stage software pipeline over chunks
            if rt == 0:
                xj_load(0)
            ss = pool_coords(0)
            fr_prev = None
            for ch in range(NCH):
                fr = front(ch, ss)
                if ch + 1 < NCH:
                    if rt == 0:
                        xj_load(ch + 1)
                    ss = pool_coords(ch + 1)
                if fr_prev is not None:
                    back(ch - 1, fr_prev)
                fr_prev = fr
            back(NCH - 1, fr_prev)

            # ---- per row-tile epilogue (tiny column math) ----
            out_sb = colp.tile([P, 4], F32, tag="osb")
            for c in range(3):
                fs = colp.tile([P, 1], F32, tag="fs")
                nc.vector.tensor_reduce(
                    fs[:], fcol[:, c * NCH : (c + 1) * NCH],
                    axis=mybir.AxisListType.X, op=OP.add,
                )
                nc.vector.tensor_scalar(
                    out_sb[:, c : c + 1], fs[:], -1.0, None, OP.mult
                )
            es = colp.tile([P, 1], F32, tag="es")
            nc.vector.tensor_reduce(
                es[:], ecol[:, :], axis=mybir.AxisListType.X, op=OP.add
            )
            dg = colp.tile([P, 5], F32, tag="dg")
            nc.sync.dma_start(dg[:, :], dgd[rs : rs + P, :])
            lnrd = colp.tile([P, 1], F32, tag="lnrd")
            nc.scalar.activation(lnrd[:], dg[:, 3:4], AT.Ln)
            rhoid = colp.tile([P, 1], F32, tag="rhoid")
            nc.scalar.activation(rhoid[:], lnrd[:], AT.Exp, scale=-1.0)
            sd = colp.tile([P, 1], F32, tag="sd")
            nc.vector.tensor_scalar(sd[:], dg[:, 4:5], -float(md), None, OP.add)
            argd = colp.tile([P, 1], F32, tag="argd")
            nc.vector.tensor_mul(argd[:], sd[:], rhoid[:])
            td = colp.tile([P, 1], F32, tag="td")
            nc.scalar.activation(td[:], argd[:], AT.Exp)
            Atd = colp.tile([P, 1], F32, tag="Atd")
            nc.vector.tensor_mul(Atd[:], dg[:, 0:1], td[:])
            ed1 = colp.tile([P, 1], F32, tag="ed1")
            nc.vector.scalar_tensor_tensor(
                ed1[:], dg[:, 1:2], -1.0, Atd[:], OP.mult, OP.add
            )
            ed = colp.tile([P, 1], F32, tag="ed")
            nc.vector.tensor_add(ed[:], ed1[:], dg[:, 2:3])
            nc.vector.scalar_tensor_tensor(
                out_sb[:, 3:4], ed[:], -1.0, es[:], OP.mult, OP.add
            )
            nc.sync.dma_start(outd[rs : rs + P, :], out_sb[:, :])

    # Force every activation to resolve into the natural_log_exp_and_others
    # table set (it contains exp, ln, square and relu): bacc's table-load
    # pass otherwise maps Exp/Square to exp_and_others and Ln to
    # natural_log_exp_and_others, inserting a ~2.7us ACT_TABLE_LOAD at every
    # transition. Presenting all other sets as empty (indices preserved)
    # makes the pass emit a single hoisted load.
    import concourse.bacc as bacc_mod

    real_get = bacc_mod.get_activation_tables

    def one_set(arch):
        tabs = real_get(arch)
        return {
            k: (v if k == "natural_log_exp_and_others" else set())
            for k, v in tabs.items()
        }

    bacc_mod.get_activation_tables = one_set
    try:
        nc.compile()
    finally:
        bacc_mod.get_activation_tables = real_get
    return nc


def _get_nc(L, md):
    key = (round(float(L), 6), round(float(md), 6))
    if key not in _cache:
        _cache[key] = _build(L, md)
    return _cache[key]


last_exec_ns = None
last_profile = None


def kernel(atom_coordinates, A, C, D, rho, sigma, box_length, cutoff, min_distance):
    import os
    from concourse.bass_utils import run_bass_kernel_spmd

    global last_exec_ns, last_profile

    coords = np.ascontiguousarray(np.asarray(atom_coordinates, dtype=np.float32))
    box = np.asarray(box_length, dtype=np.float64).reshape(-1)
    L = float(box[0])
    assert np.allclose(box, L), "kernel assumes a cubic box"
    md = float(np.asarray(min_distance).reshape(-1)[0])
    assert coords.shape == (N, 3)

    Af = np.asarray(A, dtype=np.float32)
    Cf = np.asarray(C, dtype=np.float32)
    Df = np.asarray(D, dtype=np.float32)
    Rf = np.asarray(rho, dtype=np.float32)
    Sf = np.asarray(sigma, dtype=np.float32)

    nc = _get_nc(L, md)
    xallT = np.ascontiguousarray(coords.T)
    dA, dC, dD = np.diagonal(Af), np.diagonal(Cf), np.diagonal(Df)
    dR, dS = np.diagonal(Rf), np.diagonal(Sf)

    in_maps = []
    for c in range(NCORES):
        sl = slice(c * RPC, (c + 1) * RPC)
        in_maps.append({
            "Ash": np.ascontiguousarray(Af[sl]),
            "Csh": np.ascontiguousarray(Cf[sl]),
            "Dsh": np.ascontiguousarray(Df[sl]),
            "Rsh": np.ascontiguousarray(Rf[sl]),
            "Ssh": np.ascontiguousarray(Sf[sl]),
            "xallT": xallT,
            "xown": np.ascontiguousarray(coords[sl]),
            "diagd": np.ascontiguousarray(
                np.stack([dA[sl], dC[sl], dD[sl], dR[sl], dS[sl]], axis=1)
            ).astype(np.float32),
        })

    tmpdir = os.environ.get("BMH_TRACE_DIR") or None
    r = run_bass_kernel_spmd(nc, in_maps, list(range(NCORES)), tmpdir=tmpdir)
    last_exec_ns = r.exec_time_ns
    last_profile = r.profile_json
    res = r.results
    out = np.concatenate([res[c]["out"] for c in range(NCORES)], axis=0)
    forces = np.ascontiguousarray(out[:, :3], dtype=np.float32)
    energy = np.float32(0.5 * out[:, 3].astype(np.float64).sum())
    return energy, forces


# revision 32
# speedup vs baseline: 1.0012x; 1.0012x over previous
"""Born-Mayer-Huggins pairwise energy + forces on 8 Trainium2 NeuronCores.

Row-shards the [N,N] pair matrices across 8 cores (N/8 rows each); each core
computes its force rows and per-row energy sums on device; host concatenates
forces and sums the energy.

Math (per pair i,j, minimum image under cubic PBC, box L):
    d_c   = (xj - xi)_c - L*round((xj - xi)_c / L)       (signed, = -dx_mi)
    r2    = sum_c d_c^2 ;  cl = max(r2, md^2) ;  r = sqrt(cl)
    t     = exp((sigma - r)/rho)
    e     = A*t - C*cl^-3 + D*cl^-4                      (cutoff mask dropped:
                                                          tail error ~3e-7 rel)
    g     = (A*t/rho)/r - 6*C*cl^-4 + 8*D*cl^-5
    F_i,c = -sum_j g*(r2>md^2)*d_c ;  E = (sum e - sum_diag e)/2
Diagonal energy terms (r clamped to md) are subtracted exactly per row using
the diagonal elements of A,C,D,rho,sigma.

Implementation: per 128-row tile, the 4096 pair columns are processed in
chunks as a two-stage software pipeline — "front" (minimum image, r^2,
ln/exp power chain) of chunk ch is emitted interleaved with "back"
(energy/force combine + row reductions) of chunk ch-1, so each in-order
engine stream has independent work to fill cross-engine waits.  The
minimum-image shift tiles are produced on GPSIMD, transcendentals on the
scalar engine (one activation-table set: ln/exp/square/relu), everything
else on the vector engine with fused scalar_tensor_tensor ops + free
row-sum accumulators.
"""

import numpy as np

N = 4096
NCORES = 8
RPC = N // NCORES       # rows per core
P = 128                 # SBUF partitions
RT = RPC // P           # row tiles per core
FC = 512                # free-dim chunk
NCH = N // FC           # chunks per row

_cache = {}


def _build(L, md):
    import concourse.bass as bass
    import concourse.tile as tile
    from concourse import bacc, mybir
    from contextlib import ExitStack

    AT = mybir.ActivationFunctionType
    OP = mybir.AluOpType
    F32 = mybir.dt.float32
    BF16 = mybir.dt.bfloat16
    I32 = mybir.dt.int32
    md2 = float(md) * float(md)
    relu_clamp = md2 == 1.0
    half = float(L) / 2.0

    nc = bacc.Bacc("TRN2", target_bir_lowering=False, debug=False)
    Ad = nc.dram_tensor("Ash", [RPC, N], F32, kind="ExternalInput").ap()
    Cd = nc.dram_tensor("Csh", [RPC, N], F32, kind="ExternalInput").ap()
    Dd = nc.dram_tensor("Dsh", [RPC, N], F32, kind="ExternalInput").ap()
    Rd = nc.dram_tensor("Rsh", [RPC, N], F32, kind="ExternalInput").ap()
    Sd = nc.dram_tensor("Ssh", [RPC, N], F32, kind="ExternalInput").ap()
    xTd = nc.dram_tensor("xallT", [3, N], F32, kind="ExternalInput").ap()
    xod = nc.dram_tensor("xown", [RPC, 3], F32, kind="ExternalInput").ap()
    dgd = nc.dram_tensor("diagd", [RPC, 5], F32, kind="ExternalInput").ap()
    outd = nc.dram_tensor("out", [RPC, 4], F32, kind="ExternalOutput").ap()

    with tile.TileContext(nc) as tc, ExitStack() as ctx:
        singles = ctx.enter_context(tc.tile_pool(name="singles", bufs=1))
        inpool = ctx.enter_context(tc.tile_pool(name="inp", bufs=3))
        vp = ctx.enter_context(tc.tile_pool(name="vp", bufs=2))
        gp = ctx.enter_context(tc.tile_pool(name="gp", bufs=2))
        ap2 = ctx.enter_context(tc.tile_pool(name="ap2", bufs=2))
        psum = ctx.enter_context(tc.tile_pool(name="ps", bufs=2, space="PSUM"))
        colp = ctx.enter_context(tc.tile_pool(name="col", bufs=2))

        # coordinates of all atoms, broadcast along partitions: [128, 3, N].
        # Loaded lazily per column chunk during the first row tile so early
        # compute is not gated on the full 6 MB broadcast.
        XJ = singles.tile([P, 3, N], F32)

        def xj_load(ch):
            cs = ch * FC
            for c in range(3):
                nc.sync.dma_start(
                    XJ[:, c, cs : cs + FC],
                    xTd[c : c + 1, cs : cs + FC].to_broadcast([P, FC]),
                )

        def w(tag, pool=None, dt=F32, bufs=None):
            pl = pool or vp
            return pl.tile([P, FC], dt, tag=tag, name=tag, bufs=bufs)

        # per-partition coordinate columns and thresholds, all row tiles
        cols = []
        for rt in range(RT):
            rs = rt * P
            xi = colp.tile([P, 3], F32, tag="xi", name="xi", bufs=RT)
            nc.gpsimd.dma_start(xi[:, :], xod[rs : rs + P, :])
            xp = colp.tile([P, 3], F32, tag="xp", name="xp", bufs=RT)
            nc.vector.tensor_scalar(xp[:, :], xi[:, :], half, None, OP.add)
            xm = colp.tile([P, 3], F32, tag="xm", name="xm", bufs=RT)
            nc.vector.tensor_scalar(xm[:, :], xi[:, :], -half, None, OP.add)
            xn = colp.tile([P, 3], F32, tag="xn", name="xn", bufs=RT)
            nc.vector.tensor_scalar(xn[:, :], xi[:, :], -1.0, None, OP.mult)
            xoff = colp.tile([P, 3], F32, tag="xoff", name="xoff", bufs=RT)
            nc.vector.tensor_scalar(xoff[:, :], xi[:, :], -1.0, L, OP.mult, OP.add)
            cols.append((xp, xm, xn, xoff))

        for rt in range(RT):
            rs = rt * P
            xp, xm, xn, xoff = cols[rt]

            ecol = colp.tile([P, NCH], F32, tag="ecol")
            fcol = colp.tile([P, 3 * NCH], F32, tag="fcol")

            def pool_coords(ch):
                # minimum-image shift tiles (POOL only), via the f32->int32
                # RNE conversion: k = rint((xj - xi + L)/L) in {0,1,2},
                # s = L*(1-k) in {+L, 0, -L}
                cs = ch * FC
                out = []
                for c in range(3):
                    XJc = XJ[:, c, cs : cs + FC]
                    k = w("k", gp, dt=I32, bufs=1)
                    nc.gpsimd.tensor_scalar(
                        k[:], XJc, xoff[:, c : c + 1], 1.0 / L, OP.add, OP.mult
                    )
                    s = w(f"s{c}", gp)
                    nc.gpsimd.tensor_scalar(s[:], k[:], -L, L, OP.mult, OP.add)
                    out.append(s)
                return out

            def front(ch, ss):
                cs = ch * FC
                Ri = w("R", inpool)
                nc.sync.dma_start(Ri[:], Rd[rs : rs + P, cs : cs + FC])
                ds = []
                sqs = []
                for c in range(3):
                    XJc = XJ[:, c, cs : cs + FC]
                    d = w(f"d{c}")
                    nc.vector.scalar_tensor_tensor(
                        d[:], XJc, xn[:, c : c + 1], ss[c][:], OP.add, OP.add
                    )
                    sq = w("sq0", ap2) if c == 0 else w(f"sq{c}", psum)
                    nc.scalar.activation(sq[:], d[:], AT.Square)
                    ds.append(d)
                    sqs.append(sq)
                r2a = w("r2a")
                nc.vector.tensor_add(r2a[:], sqs[0][:], sqs[1][:])
                r2 = w("r2")
                nc.vector.tensor_add(r2[:], r2a[:], sqs[2][:])
                if relu_clamp:
                    # ln(max(r2,1)) == relu(ln(r2)); ln(0)=-inf -> relu -> 0
                    lr0 = w("lr0", ap2)
                    nc.scalar.activation(lr0[:], r2[:], AT.Ln)
                    lr = w("lrc", ap2)
                    nc.scalar.activation(lr[:], lr0[:], AT.Relu)
                else:
                    cl = w("lr0", ap2)
                    nc.vector.tensor_scalar(cl[:], r2[:], md2, None, OP.max)
                    lr = w("lrc", ap2)
                    nc.scalar.activation(lr[:], cl[:], AT.Ln)
                r_ = w("r_", ap2)
                nc.scalar.activation(r_[:], lr[:], AT.Exp, scale=0.5)
                lnr = w("lnr", ap2)
                nc.scalar.activation(lnr[:], Ri[:], AT.Ln)
                rhoi = w("rhoi", ap2)
                nc.scalar.activation(rhoi[:], lnr[:], AT.Exp, scale=-1.0)
                rinv = w("rinv", ap2)
                nc.scalar.activation(rinv[:], lr[:], AT.Exp, scale=-0.5)
                r2i = w("r2i", ap2)
                nc.scalar.activation(r2i[:], lr[:], AT.Exp, scale=-1.0)
                r6i = w("r6i", ap2)
                nc.scalar.activation(r6i[:], lr[:], AT.Exp, scale=-3.0)
                r8i = w("r8i", ap2)
                nc.scalar.activation(r8i[:], lr[:], AT.Exp, scale=-4.0)
                rr = w("rr", gp)
                nc.gpsimd.tensor_tensor(rr[:], rhoi[:], rinv[:], OP.mult)
                return dict(ds=ds, r2=r2, r_=r_, rinv=rinv, r2i=r2i,
                            r6i=r6i, r8i=r8i, rhoi=rhoi, rr=rr)

            def back(ch, f):
                cs = ch * FC
                Ai = w("A", inpool)
                nc.sync.dma_start(Ai[:], Ad[rs : rs + P, cs : cs + FC])
                Ci = w("C", inpool)
                nc.sync.dma_start(Ci[:], Cd[rs : rs + P, cs : cs + FC])
                Di = w("D", inpool)
                nc.sync.dma_start(Di[:], Dd[rs : rs + P, cs : cs + FC])
                Si = w("S", inpool)
                nc.sync.dma_start(Si[:], Sd[rs : rs + P, cs : cs + FC])

                diff = w("v1")
                nc.vector.scalar_tensor_tensor(
                    diff[:], f["r_"][:], -1.0, Si[:], OP.mult, OP.add
                )
                arg = w("arg", ap2)
                nc.vector.tensor_mul(arg[:], diff[:], f["rhoi"][:])
                t_ = w("t", psum)
                nc.scalar.activation(t_[:], arg[:], AT.Exp)

                At = w("v2")
                nc.vector.tensor_mul(At[:], Ai[:], t_[:])
                P1 = w("v3")
                nc.vector.tensor_mul(P1[:], At[:], f["rr"][:])
                Du = w("Du", gp)
                nc.gpsimd.tensor_tensor(Du[:], Di[:], f["r2i"][:], OP.mult)
                CDe = w("v4")
                nc.vector.scalar_tensor_tensor(
                    CDe[:], Du[:], -1.0, Ci[:], OP.mult, OP.add
                )
                pe = w("pe", gp)
                nc.gpsimd.tensor_tensor(pe[:], f["r6i"][:], CDe[:], OP.mult)
                junk = w("junk", bufs=1)
                nc.vector.scalar_tensor_tensor(
                    junk[:], pe[:], -1.0, At[:], OP.mult, OP.add,
                    accum_out=ecol[:, ch : ch + 1],
                )
                CDg = w("v2")
                nc.vector.scalar_tensor_tensor(
                    CDg[:], Du[:], -4.0 / 3.0, Ci[:], OP.mult, OP.add
                )
                prod = w("v4")
                nc.vector.tensor_mul(prod[:], f["r8i"][:], CDg[:])
                g = w("v1")
                nc.vector.scalar_tensor_tensor(
                    g[:], prod[:], -6.0, P1[:], OP.mult, OP.add
                )
                gm = w("v3")
                nc.vector.scalar_tensor_tensor(
                    gm[:], f["r2"][:], md2, g[:], OP.is_gt, OP.mult
                )
                for c in range(3):
                    junk2 = w("junk", bufs=1)
                    nc.vector.scalar_tensor_tensor(
                        junk2[:], gm[:], 0.0, f["ds"][c][:], OP.add, OP.mult,
                        accum_out=fcol[:, c * NCH + ch : c * NCH + ch + 1],
                    )

            # two-stage software pipeline over chunks
            if rt == 0:
                xj_load(0)
            ss = pool_coords(0)
            fr_prev = None
            for ch in range(NCH):
                fr = front(ch, ss)
                if ch + 1 < NCH:
                    if rt == 0:
                        xj_load(ch + 1)
                    ss = pool_coords(ch + 1)
                if fr_prev is not None:
                    back(ch - 1, fr_prev)
                fr_prev = fr
            back(NCH - 1, fr_prev)

            # ---- per row-tile epilogue (tiny column math) ----
            out_sb = colp.tile([P, 4], F32, tag="osb")
            for c in range(3):
                fs = colp.tile([P, 1], F32, tag="fs")
                nc.vector.tensor_reduce(
                    fs[:], fcol[:, c * NCH : (c + 1) * NCH],
                    axis=mybir.AxisListType.X, op=OP.add,
                )
                nc.vector.tensor_scalar(
                    out_sb[:, c : c + 1], fs[:], -1.0, None, OP.mult
                )
            es = colp.tile([P, 1], F32, tag="es")
            nc.vector.tensor_reduce(
                es[:], ecol[:, :], axis=mybir.AxisListType.X, op=OP.add
            )
            dg = colp.tile([P, 5], F32, tag="dg")
            nc.sync.dma_start(dg[:, :], dgd[rs : rs + P, :])
            lnrd = colp.tile([P, 1], F32, tag="lnrd")
            nc.scalar.activation(lnrd[:], dg[:, 3:4], AT.Ln)
            rhoid = colp.tile([P, 1], F32, tag="rhoid")
            nc.scalar.activation(rhoid[:], lnrd[:], AT.Exp, scale=-1.0)
            sd = colp.tile([P, 1], F32, tag="sd")
            nc.vector.tensor_scalar(sd[:], dg[:, 4:5], -float(md), None, OP.add)
            argd = colp.tile([P, 1], F32, tag="argd")
            nc.vector.tensor_mul(argd[:], sd[:], rhoid[:])
            td = colp.tile([P, 1], F32, tag="td")
            nc.scalar.activation(td[:], argd[:], AT.Exp)
            Atd = colp.tile([P, 1], F32, tag="Atd")
            nc.vector.tensor_mul(Atd[:], dg[:, 0:1], td[:])
            ed1 = colp.tile([P, 1], F32, tag="ed1")
            nc.vector.scalar_tensor_tensor(
                ed1[:], dg[:, 1:2], -1.0, Atd[:], OP.mult, OP.add
            )
            ed = colp.tile([P, 1], F32, tag="ed")
            nc.vector.tensor_add(ed[:], ed1[:], dg[:, 2:3])
            nc.vector.scalar_tensor_tensor(
                out_sb[:, 3:4], ed[:], -1.0, es[:], OP.mult, OP.add
            )
            nc.sync.dma_start(outd[rs : rs + P, :], out_sb[:, :])

    # Force every activation to resolve into the natural_log_exp_and_others
    # table set (it contains exp, ln, square and relu): bacc's table-load
    # pass otherwise maps Exp/Square to exp_and_others and Ln to
    # natural_log_exp_and_others, inserting a ~2.7us ACT_TABLE_LOAD at every
    # transition. Presenting all other sets as empty (indices preserved)
    # makes the pass emit a single hoisted load.
    import concourse.bacc as bacc_mod

    real_get = bacc_mod.get_activation_tables

    def one_set(arch):
        tabs = real_get(arch)
        return {
            k: (v if k == "natural_log_exp_and_others" else set())
            for k, v in tabs.items()
        }

    bacc_mod.get_activation_tables = one_set
    try:
        nc.compile()
    finally:
        bacc_mod.get_activation_tables = real_get
    return nc


def _get_nc(L, md):
    key = (round(float(L), 6), round(float(md), 6))
    if key not in _cache:
        _cache[key] = _build(L, md)
    return _cache[key]


last_exec_ns = None
last_profile = None


def kernel(atom_coordinates, A, C, D, rho, sigma, box_length, cutoff, min_distance):
    import os
    from concourse.bass_utils import run_bass_kernel_spmd

    global last_exec_ns, last_profile

    coords = np.ascontiguousarray(np.asarray(atom_coordinates, dtype=np.float32))
    box = np.asarray(box_length, dtype=np.float64).reshape(-1)
    L = float(box[0])
    assert np.allclose(box, L), "kernel assumes a cubic box"
    md = float(np.asarray(min_distance).reshape(-1)[0])
    assert coords.shape == (N, 3)

    Af = np.asarray(A, dtype=np.float32)
    Cf = np.asarray(C, dtype=np.float32)
    Df = np.asarray(D, dtype=np.float32)
    Rf = np.asarray(rho, dtype=np.float32)
    Sf = np.asarray(sigma, dtype=np.float32)

    nc = _get_nc(L, md)
    xallT = np.ascontiguousarray(coords.T)
    dA, dC, dD = np.diagonal(Af), np.diagonal(Cf), np.diagonal(Df)
    dR, dS = np.diagonal(Rf), np.diagonal(Sf)

    in_maps = []
    for c in range(NCORES):
        sl = slice(c * RPC, (c + 1) * RPC)
        in_maps.append({
            "Ash": np.ascontiguousarray(Af[sl]),
            "Csh": np.ascontiguousarray(Cf[sl]),
            "Dsh": np.ascontiguousarray(Df[sl]),
            "Rsh": np.ascontiguousarray(Rf[sl]),
            "Ssh": np.ascontiguousarray(Sf[sl]),
            "xallT": xallT,
            "xown": np.ascontiguousarray(coords[sl]),
            "diagd": np.ascontiguousarray(
                np.stack([dA[sl], dC[sl], dD[sl], dR[sl], dS[sl]], axis=1)
            ).astype(np.float32),
        })

    tmpdir = os.environ.get("BMH_TRACE_DIR") or None
    r = run_bass_kernel_spmd(nc, in_maps, list(range(NCORES)), tmpdir=tmpdir)
    last_exec_ns = r.exec_time_ns
    last_profile = r.profile_json
    res = r.results
    out = np.concatenate([res[c]["out"] for c in range(NCORES)], axis=0)
    forces = np.ascontiguousarray(out[:, :3], dtype=np.float32)
    energy = np.float32(0.5 * out[:, 3].astype(np.float64).sum())
    return energy, forces


# revision 36
# speedup vs baseline: 1.0131x; 1.0119x over previous
"""Born-Mayer-Huggins pairwise energy + forces on 8 Trainium2 NeuronCores.

Row-shards the [N,N] pair matrices across 8 cores (N/8 rows each); each core
computes its force rows and per-row energy sums on device; host concatenates
forces and sums the energy.

Math (per pair i,j, minimum image under cubic PBC, box L):
    d_c   = (xj - xi)_c - L*round((xj - xi)_c / L)       (signed, = -dx_mi)
    r2    = sum_c d_c^2 ;  cl = max(r2, md^2) ;  r = sqrt(cl)
    t     = exp((sigma - r)/rho)
    e     = A*t - C*cl^-3 + D*cl^-4                      (cutoff mask dropped:
                                                          tail error ~3e-7 rel)
    g     = (A*t/rho)/r - 6*C*cl^-4 + 8*D*cl^-5
    F_i,c = -sum_j g*(r2>md^2)*d_c ;  E = (sum e - sum_diag e)/2
Diagonal energy terms (r clamped to md) are subtracted exactly per row using
the diagonal elements of A,C,D,rho,sigma.

Implementation: per 128-row tile, the 4096 pair columns are processed in
chunks as a two-stage software pipeline — "front" (minimum image, r^2,
ln/exp power chain) of chunk ch is emitted interleaved with "back"
(energy/force combine + row reductions) of chunk ch-1, so each in-order
engine stream has independent work to fill cross-engine waits.  The
minimum-image shift tiles are produced on GPSIMD, transcendentals on the
scalar engine (one activation-table set: ln/exp/square/relu), everything
else on the vector engine with fused scalar_tensor_tensor ops + free
row-sum accumulators.
"""

import numpy as np

N = 4096
NCORES = 8
RPC = N // NCORES       # rows per core
P = 128                 # SBUF partitions
RT = RPC // P           # row tiles per core
FC = 512                # free-dim chunk
NCH = N // FC           # chunks per row

_cache = {}


def _build(L, md):
    import concourse.bass as bass
    import concourse.tile as tile
    from concourse import bacc, mybir
    from contextlib import ExitStack

    AT = mybir.ActivationFunctionType
    OP = mybir.AluOpType
    F32 = mybir.dt.float32
    BF16 = mybir.dt.bfloat16
    I32 = mybir.dt.int32
    md2 = float(md) * float(md)
    relu_clamp = md2 == 1.0
    half = float(L) / 2.0

    nc = bacc.Bacc("TRN2", target_bir_lowering=False, debug=False)
    Ad = nc.dram_tensor("Ash", [RPC, N], F32, kind="ExternalInput").ap()
    Cd = nc.dram_tensor("Csh", [RPC, N], F32, kind="ExternalInput").ap()
    Dd = nc.dram_tensor("Dsh", [RPC, N], F32, kind="ExternalInput").ap()
    Rd = nc.dram_tensor("Rsh", [RPC, N], F32, kind="ExternalInput").ap()
    Sd = nc.dram_tensor("Ssh", [RPC, N], F32, kind="ExternalInput").ap()
    xTd = nc.dram_tensor("xallT", [3, N], F32, kind="ExternalInput").ap()
    xod = nc.dram_tensor("xown", [RPC, 3], F32, kind="ExternalInput").ap()
    dgd = nc.dram_tensor("diagd", [RPC, 5], F32, kind="ExternalInput").ap()
    outd = nc.dram_tensor("out", [RPC, 4], F32, kind="ExternalOutput").ap()

    with tile.TileContext(nc) as tc, ExitStack() as ctx:
        singles = ctx.enter_context(tc.tile_pool(name="singles", bufs=1))
        inpool = ctx.enter_context(tc.tile_pool(name="inp", bufs=3))
        vp = ctx.enter_context(tc.tile_pool(name="vp", bufs=2))
        gp = ctx.enter_context(tc.tile_pool(name="gp", bufs=2))
        ap2 = ctx.enter_context(tc.tile_pool(name="ap2", bufs=2))
        psum = ctx.enter_context(tc.tile_pool(name="ps", bufs=2, space="PSUM"))
        colp = ctx.enter_context(tc.tile_pool(name="col", bufs=2))

        # coordinates of all atoms, broadcast along partitions: [128, 3, N].
        # Loaded lazily per column chunk during the first row tile so early
        # compute is not gated on the full 6 MB broadcast.
        XJ = singles.tile([P, 3, N], F32)

        def xj_load(ch):
            cs = ch * FC
            for c in range(3):
                nc.sync.dma_start(
                    XJ[:, c, cs : cs + FC],
                    xTd[c : c + 1, cs : cs + FC].to_broadcast([P, FC]),
                )

        def w(tag, pool=None, dt=F32, bufs=None):
            pl = pool or vp
            return pl.tile([P, FC], dt, tag=tag, name=tag, bufs=bufs)

        # per-partition coordinate columns and thresholds, all row tiles
        cols = []
        for rt in range(RT):
            rs = rt * P
            xi = colp.tile([P, 3], F32, tag="xi", name="xi", bufs=RT)
            nc.gpsimd.dma_start(xi[:, :], xod[rs : rs + P, :])
            xp = colp.tile([P, 3], F32, tag="xp", name="xp", bufs=RT)
            nc.vector.tensor_scalar(xp[:, :], xi[:, :], half, None, OP.add)
            xm = colp.tile([P, 3], F32, tag="xm", name="xm", bufs=RT)
            nc.vector.tensor_scalar(xm[:, :], xi[:, :], -half, None, OP.add)
            xn = colp.tile([P, 3], F32, tag="xn", name="xn", bufs=RT)
            nc.vector.tensor_scalar(xn[:, :], xi[:, :], -1.0, None, OP.mult)
            xoff = colp.tile([P, 3], F32, tag="xoff", name="xoff", bufs=RT)
            nc.vector.tensor_scalar(xoff[:, :], xi[:, :], -1.0, L, OP.mult, OP.add)
            cols.append((xp, xm, xn, xoff))

        for rt in range(RT):
            rs = rt * P
            xp, xm, xn, xoff = cols[rt]

            ecol = colp.tile([P, NCH], F32, tag="ecol")
            fcol = colp.tile([P, 3 * NCH], F32, tag="fcol")

            def pool_coords(ch):
                # minimum-image shift tiles (POOL only), via the f32->int32
                # RNE conversion: k = rint((xj - xi + L)/L) in {0,1,2},
                # s = L*(1-k) in {+L, 0, -L}
                cs = ch * FC
                out = []
                for c in range(3):
                    XJc = XJ[:, c, cs : cs + FC]
                    k = w("k", gp, dt=I32, bufs=1)
                    nc.gpsimd.tensor_scalar(
                        k[:], XJc, xoff[:, c : c + 1], 1.0 / L, OP.add, OP.mult
                    )
                    s = w(f"s{c}", gp)
                    nc.gpsimd.tensor_scalar(s[:], k[:], -L, L, OP.mult, OP.add)
                    out.append(s)
                return out

            def front(ch, ss):
                cs = ch * FC
                Ri = w("R", inpool)
                nc.sync.dma_start(Ri[:], Rd[rs : rs + P, cs : cs + FC])
                ds = []
                sqs = []
                for c in range(3):
                    XJc = XJ[:, c, cs : cs + FC]
                    d = w(f"d{c}")
                    nc.vector.scalar_tensor_tensor(
                        d[:], XJc, xn[:, c : c + 1], ss[c][:], OP.add, OP.add
                    )
                    sq = w("sq0", ap2) if c == 0 else w(f"sq{c}", psum)
                    nc.scalar.activation(sq[:], d[:], AT.Square)
                    ds.append(d)
                    sqs.append(sq)
                r2a = w("r2a")
                nc.vector.tensor_add(r2a[:], sqs[0][:], sqs[1][:])
                r2 = w("r2")
                nc.vector.tensor_add(r2[:], r2a[:], sqs[2][:])
                if relu_clamp:
                    # ln(max(r2,1)) == relu(ln(r2)); ln(0)=-inf -> relu -> 0
                    lr0 = w("lr0", ap2)
                    nc.scalar.activation(lr0[:], r2[:], AT.Ln)
                    lr = w("lrc", ap2)
                    nc.scalar.activation(lr[:], lr0[:], AT.Relu)
                else:
                    cl = w("lr0", ap2)
                    nc.vector.tensor_scalar(cl[:], r2[:], md2, None, OP.max)
                    lr = w("lrc", ap2)
                    nc.scalar.activation(lr[:], cl[:], AT.Ln)
                r_ = w("r_", ap2)
                nc.scalar.activation(r_[:], lr[:], AT.Exp, scale=0.5)
                lnr = w("lnr", ap2)
                nc.scalar.activation(lnr[:], Ri[:], AT.Ln)
                rhoi = w("rhoi", ap2)
                nc.scalar.activation(rhoi[:], lnr[:], AT.Exp, scale=-1.0)
                rinv = w("rinv", ap2)
                nc.scalar.activation(rinv[:], lr[:], AT.Exp, scale=-0.5)
                r2i = w("r2i", ap2)
                nc.scalar.activation(r2i[:], lr[:], AT.Exp, scale=-1.0)
                r6i = w("r6i", ap2)
                nc.scalar.activation(r6i[:], lr[:], AT.Exp, scale=-3.0)
                r8i = w("r8i", ap2)
                nc.scalar.activation(r8i[:], lr[:], AT.Exp, scale=-4.0)
                rr = w("rr", gp)
                nc.gpsimd.tensor_tensor(rr[:], rhoi[:], rinv[:], OP.mult)
                return dict(ds=ds, r2=r2, r_=r_, rinv=rinv, r2i=r2i,
                            r6i=r6i, r8i=r8i, rhoi=rhoi, rr=rr)

            def back(ch, f):
                cs = ch * FC
                Ai = w("A", inpool)
                nc.sync.dma_start(Ai[:], Ad[rs : rs + P, cs : cs + FC])
                Ci = w("C", inpool)
                nc.sync.dma_start(Ci[:], Cd[rs : rs + P, cs : cs + FC])
                Di = w("D", inpool)
                nc.sync.dma_start(Di[:], Dd[rs : rs + P, cs : cs + FC])
                Si = w("S", inpool)
                nc.sync.dma_start(Si[:], Sd[rs : rs + P, cs : cs + FC])

                diff = w("v1")
                nc.vector.scalar_tensor_tensor(
                    diff[:], f["r_"][:], -1.0, Si[:], OP.mult, OP.add
                )
                arg = w("arg", ap2)
                nc.vector.tensor_mul(arg[:], diff[:], f["rhoi"][:])
                t_ = w("t", psum)
                nc.scalar.activation(t_[:], arg[:], AT.Exp)

                At = w("v2")
                nc.vector.tensor_mul(At[:], Ai[:], t_[:])
                P1 = w("v3")
                nc.vector.tensor_mul(P1[:], At[:], f["rr"][:])
                Du = w("Du", gp)
                nc.gpsimd.tensor_tensor(Du[:], Di[:], f["r2i"][:], OP.mult)
                CDe = w("v4")
                nc.vector.scalar_tensor_tensor(
                    CDe[:], Du[:], -1.0, Ci[:], OP.mult, OP.add
                )
                pe = w("pe", gp)
                nc.gpsimd.tensor_tensor(pe[:], f["r6i"][:], CDe[:], OP.mult)
                junk = w("junk", bufs=1)
                nc.vector.scalar_tensor_tensor(
                    junk[:], pe[:], -1.0, At[:], OP.mult, OP.add,
                    accum_out=ecol[:, ch : ch + 1],
                )
                CDg = w("v2")
                nc.vector.scalar_tensor_tensor(
                    CDg[:], Du[:], -4.0 / 3.0, Ci[:], OP.mult, OP.add
                )
                prod = w("v4")
                nc.vector.tensor_mul(prod[:], f["r8i"][:], CDg[:])
                g = w("v1")
                nc.vector.scalar_tensor_tensor(
                    g[:], prod[:], -6.0, P1[:], OP.mult, OP.add
                )
                gm = w("v3")
                nc.vector.scalar_tensor_tensor(
                    gm[:], f["r2"][:], md2, g[:], OP.is_gt, OP.mult
                )
                for c in range(3):
                    junk2 = w("junk", bufs=1)
                    nc.vector.scalar_tensor_tensor(
                        junk2[:], gm[:], 0.0, f["ds"][c][:], OP.add, OP.mult,
                        accum_out=fcol[:, c * NCH + ch : c * NCH + ch + 1],
                    )

            # two-stage software pipeline over chunks
            if rt == 0:
                xj_load(0)
            ss = pool_coords(0)
            fr_prev = None
            for ch in range(NCH):
                fr = front(ch, ss)
                if ch + 1 < NCH:
                    if rt == 0:
                        xj_load(ch + 1)
                    ss = pool_coords(ch + 1)
                if fr_prev is not None:
                    back(ch - 1, fr_prev)
                fr_prev = fr
            back(NCH - 1, fr_prev)

            # ---- per row-tile epilogue (tiny column math) ----
            out_sb = colp.tile([P, 4], F32, tag="osb")
            for c in range(3):
                fs = colp.tile([P, 1], F32, tag="fs")
                nc.vector.tensor_reduce(
                    fs[:], fcol[:, c * NCH : (c + 1) * NCH],
                    axis=mybir.AxisListType.X, op=OP.add,
                )
                nc.vector.tensor_scalar(
                    out_sb[:, c : c + 1], fs[:], -1.0, None, OP.mult
                )
            es = colp.tile([P, 1], F32, tag="es")
            nc.vector.tensor_reduce(
                es[:], ecol[:, :], axis=mybir.AxisListType.X, op=OP.add
            )
            dg = colp.tile([P, 5], F32, tag="dg")
            nc.sync.dma_start(dg[:, :], dgd[rs : rs + P, :])
            lnrd = colp.tile([P, 1], F32, tag="lnrd")
            nc.scalar.activation(lnrd[:], dg[:, 3:4], AT.Ln)
            rhoid = colp.tile([P, 1], F32, tag="rhoid")
            nc.scalar.activation(rhoid[:], lnrd[:], AT.Exp, scale=-1.0)
            sd = colp.tile([P, 1], F32, tag="sd")
            nc.vector.tensor_scalar(sd[:], dg[:, 4:5], -float(md), None, OP.add)
            argd = colp.tile([P, 1], F32, tag="argd")
            nc.vector.tensor_mul(argd[:], sd[:], rhoid[:])
            td = colp.tile([P, 1], F32, tag="td")
            nc.scalar.activation(td[:], argd[:], AT.Exp)
            Atd = colp.tile([P, 1], F32, tag="Atd")
            nc.vector.tensor_mul(Atd[:], dg[:, 0:1], td[:])
            ed1 = colp.tile([P, 1], F32, tag="ed1")
            nc.vector.scalar_tensor_tensor(
                ed1[:], dg[:, 1:2], -1.0, Atd[:], OP.mult, OP.add
            )
            ed = colp.tile([P, 1], F32, tag="ed")
            nc.vector.tensor_add(ed[:], ed1[:], dg[:, 2:3])
            nc.vector.scalar_tensor_tensor(
                out_sb[:, 3:4], ed[:], -1.0, es[:], OP.mult, OP.add
            )
            nc.sync.dma_start(outd[rs : rs + P, :], out_sb[:, :])

    # Force every activation to resolve into the natural_log_exp_and_others
    # table set (it contains exp, ln, square and relu): bacc's table-load
    # pass otherwise maps Exp/Square to exp_and_others and Ln to
    # natural_log_exp_and_others, inserting a ~2.7us ACT_TABLE_LOAD at every
    # transition. Presenting all other sets as empty (indices preserved)
    # makes the pass emit a single hoisted load.
    import concourse.bacc as bacc_mod

    real_get = bacc_mod.get_activation_tables

    def one_set(arch):
        tabs = real_get(arch)
        return {
            k: (v if k == "natural_log_exp_and_others" else set())
            for k, v in tabs.items()
        }

    bacc_mod.get_activation_tables = one_set
    try:
        nc.compile()
    finally:
        bacc_mod.get_activation_tables = real_get
    return nc


def _get_nc(L, md):
    key = (round(float(L), 6), round(float(md), 6))
    if key not in _cache:
        _cache[key] = _build(L, md)
    return _cache[key]


last_exec_ns = None
last_profile = None


def kernel(atom_coordinates, A, C, D, rho, sigma, box_length, cutoff, min_distance):
    import os
    from concourse.bass_utils import run_bass_kernel_spmd

    global last_exec_ns, last_profile

    coords = np.ascontiguousarray(np.asarray(atom_coordinates, dtype=np.float32))
    box = np.asarray(box_length, dtype=np.float64).reshape(-1)
    L = float(box[0])
    assert np.allclose(box, L), "kernel assumes a cubic box"
    md = float(np.asarray(min_distance).reshape(-1)[0])
    assert coords.shape == (N, 3)

    Af = np.asarray(A, dtype=np.float32)
    Cf = np.asarray(C, dtype=np.float32)
    Df = np.asarray(D, dtype=np.float32)
    Rf = np.asarray(rho, dtype=np.float32)
    Sf = np.asarray(sigma, dtype=np.float32)

    nc = _get_nc(L, md)
    xallT = np.ascontiguousarray(coords.T)
    dA, dC, dD = np.diagonal(Af), np.diagonal(Cf), np.diagonal(Df)
    dR, dS = np.diagonal(Rf), np.diagonal(Sf)

    in_maps = []
    for c in range(NCORES):
        sl = slice(c * RPC, (c + 1) * RPC)
        in_maps.append({
            "Ash": np.ascontiguousarray(Af[sl]),
            "Csh": np.ascontiguousarray(Cf[sl]),
            "Dsh": np.ascontiguousarray(Df[sl]),
            "Rsh": np.ascontiguousarray(Rf[sl]),
            "Ssh": np.ascontiguousarray(Sf[sl]),
            "xallT": xallT,
            "xown": np.ascontiguousarray(coords[sl]),
            "diagd": np.ascontiguousarray(
                np.stack([dA[sl], dC[sl], dD[sl], dR[sl], dS[sl]], axis=1)
            ).astype(np.float32),
        })

    tmpdir = os.environ.get("BMH_TRACE_DIR") or None
    r = run_bass_kernel_spmd(nc, in_maps, list(range(NCORES)), tmpdir=tmpdir)
    last_exec_ns = r.exec_time_ns
    last_profile = r.profile_json
    res = r.results
    out = np.concatenate([res[c]["out"] for c in range(NCORES)], axis=0)
    forces = np.ascontiguousarray(out[:, :3], dtype=np.float32)
    energy = np.float32(0.5 * out[:, 3].astype(np.float64).sum())
    return energy, forces
